# revision 2
# baseline (speedup 1.0000x reference)
"""Trainium2 Bass kernel for nn_STContrastiveReIDLoss (B=8192, D=2048, C=751).

Strategy (8 NeuronCores, SPMD, no collectives):
  - Shard batch rows across cores (1024 anchors/core); every core streams the
    full feature set as the RHS of a gram matmul, so all B x B pairwise
    quantities for its anchor rows are computed locally.
  - One fp16 gram matmul G = f_local @ f_all^T drives all three losses:
      * triplet:   d2_ij = sq_i + sq_j - 2 G_ij   (hardest_neg == 0 analytically,
                   since the reference's neg mask keeps the diagonal and d2_ii = 0)
      * st-InfoNCE: sim_ij = G_ij * u_i * u_j / TEMP  (u = 1/||f||)
      * id loss:    separate fp16 matmul vs W^T (bias folded in via K+1 row)
  - Label equality masks via fp16 compares on-chip; camera reachability
    threshold via a K=16 one-hot matmul th_ij = reach_max[cam_i, cam_j].
  - Online (rescaled) softmax accumulation over 512-column chunks, so no
    B-wide intermediate is ever materialized.
  - Per-row partial stats are written out ([128, 8, 8] fp32 per core); the
    final scalar reduction (logs, divisions, diagonal corrections) runs on
    host in float64.
"""

import numpy as np

import concourse.bacc as bacc
import concourse.bass as bass
import concourse.mybir as mybir
from concourse.alu_op_type import AluOpType
from concourse.bass_utils import run_bass_kernel_spmd
from concourse.tile import TileContext

B, D, C = 8192, 2048, 751
NCAMS = 16
MARGIN = 0.3
TEMP = 0.07
L_TRI = 0.5
L_ST = 0.3

NCORES = 8
BLOC = B // NCORES          # rows per core (1024)
RB = BLOC // 128            # row-blocks per core (8)
NJ = 512                    # column chunk width
JC = B // NJ                # column chunks (16)
KT = D // 128               # contraction k-tiles (16)
KTA = KT + 1                # + bias row tile for the classifier
MON_INIT = -60000.0         # "-inf" for the online max (fp32)
TRI_VALID_THRESH = 1000.0   # mtri above this => anchor has a real positive

f16 = np.float16
f32 = np.float32
dt = mybir.dt
AF = mybir.ActivationFunctionType

_NC_CACHE = {}


def _build_nc():
    nc = bacc.Bacc("TRN2", target_bir_lowering=False, debug=False)

    d_fta = nc.dram_tensor("fta", [KT, JC, 128, NJ], dt.float16, kind="ExternalInput")
    d_ftaloc = nc.dram_tensor("ftaloc", [KTA, 2, 128, NJ], dt.float16, kind="ExternalInput")
    d_wta = nc.dram_tensor("wta", [KTA, 128, C], dt.float16, kind="ExternalInput")
    d_labv = nc.dram_tensor("labv", [B], dt.float16, kind="ExternalInput")
    d_tsv = nc.dram_tensor("tsv", [B], dt.float16, kind="ExternalInput")
    d_uv = nc.dram_tensor("uv", [B], dt.float16, kind="ExternalInput")
    d_sqv = nc.dram_tensor("sqv", [B], dt.float16, kind="ExternalInput")
    d_camoh = nc.dram_tensor("camoh", [NCAMS, B], dt.float16, kind="ExternalInput")
    d_rsel = nc.dram_tensor("rsel", [NCAMS, BLOC], dt.float16, kind="ExternalInput")
    d_loh = nc.dram_tensor("loh", [RB, 128, C], dt.float16, kind="ExternalInput")
    d_labi = nc.dram_tensor("labi", [128, RB], dt.float32, kind="ExternalInput")
    d_ntsi = nc.dram_tensor("ntsi", [128, RB], dt.float32, kind="ExternalInput")
    d_ai = nc.dram_tensor("ai", [128, RB], dt.float32, kind="ExternalInput")
    d_stats = nc.dram_tensor("stats", [128, RB, 8], dt.float32, kind="ExternalOutput")

    def bcast(dram_vec, off, n):
        return bass.AP(tensor=dram_vec, offset=off, ap=[[0, 128], [1, n]])

    with TileContext(nc) as tc:
        with (
            tc.tile_pool(name="const", bufs=1) as cpool,
            tc.tile_pool(name="accs", bufs=1) as apool,
            tc.tile_pool(name="rhs", bufs=2) as rpool,
            tc.tile_pool(name="vecs", bufs=2) as vpool,
            tc.tile_pool(name="loh", bufs=2) as lpool,
            tc.tile_pool(name="scr", bufs=3) as spool,
            tc.tile_pool(name="side", bufs=6) as dpool,
            tc.tile_pool(name="psg", bufs=2, space="PSUM") as psg,
            tc.tile_pool(name="psth", bufs=2, space="PSUM") as psth,
            tc.tile_pool(name="pscls", bufs=2, space="PSUM") as pscls,
        ):
            # ---- resident constants ----
            fta_loc = cpool.tile([128, KTA, BLOC], dt.float16)
            for k in range(KTA):
                for h in range(2):
                    nc.sync.dma_start(
                        out=fta_loc[:, k, h * NJ:(h + 1) * NJ], in_=d_ftaloc[k, h]
                    )
            wta_s = cpool.tile([128, KTA, C], dt.float16)
            for k in range(KTA):
                nc.sync.dma_start(out=wta_s[:, k, :], in_=d_wta[k])
            camoh_s = cpool.tile([NCAMS, B], dt.float16)
            nc.sync.dma_start(out=camoh_s, in_=d_camoh[:, :])
            rsel_s = cpool.tile([NCAMS, BLOC], dt.float16)
            nc.sync.dma_start(out=rsel_s, in_=d_rsel[:, :])
            labi_s = cpool.tile([128, RB], dt.float32)
            nc.sync.dma_start(out=labi_s, in_=d_labi[:, :])
            ntsi_s = cpool.tile([128, RB], dt.float32)
            nc.sync.dma_start(out=ntsi_s, in_=d_ntsi[:, :])
            ai_s = cpool.tile([128, RB], dt.float32)
            nc.sync.dma_start(out=ai_s, in_=d_ai[:, :])

            # ---- accumulators ----
            npos_acc = apool.tile([128, RB, JC], dt.float32)
            p_acc = apool.tile([128, RB, JC], dt.float32)
            mtri_acc = apool.tile([128, RB, JC], dt.float32)
            mon = apool.tile([128, RB * 2], dt.float32)   # online max ping-pong
            zon = apool.tile([128, RB * 2], dt.float32)   # online sum-exp ping-pong
            stats_s = apool.tile([128, RB, 8], dt.float32)
            nc.vector.memset(mon, MON_INIT)
            nc.vector.memset(zon, 0.0)

            # ---- main loop over column chunks ----
            for jc in range(JC):
                rhs_t = rpool.tile([128, KT, NJ], dt.float16)
                for k in range(KT):
                    nc.sync.dma_start(out=rhs_t[:, k, :], in_=d_fta[k, jc])
                labr = vpool.tile([128, NJ], dt.float16)
                nc.sync.dma_start(out=labr, in_=bcast(d_labv, jc * NJ, NJ))
                tsr = vpool.tile([128, NJ], dt.float16)
                nc.sync.dma_start(out=tsr, in_=bcast(d_tsv, jc * NJ, NJ))
                ur = vpool.tile([128, NJ], dt.float16)
                nc.sync.dma_start(out=ur, in_=bcast(d_uv, jc * NJ, NJ))
                sqr = vpool.tile([128, NJ], dt.float16)
                nc.sync.dma_start(out=sqr, in_=bcast(d_sqv, jc * NJ, NJ))

                for rb in range(RB):
                    rsl = slice(rb * 128, (rb + 1) * 128)
                    lab_i = labi_s[:, rb:rb + 1]
                    nts_i = ntsi_s[:, rb:rb + 1]
                    a_i = ai_s[:, rb:rb + 1]

                    g_ps = psg.tile([128, NJ], dt.float32)
                    for k in range(KT):
                        nc.tensor.matmul(
                            out=g_ps, lhsT=fta_loc[:, k, rsl], rhs=rhs_t[:, k, :],
                            start=(k == 0), stop=(k == KT - 1),
                        )
                    th_ps = psth.tile([128, NJ], dt.float32)
                    nc.tensor.matmul(
                        out=th_ps, lhsT=rsel_s[:, rsl],
                        rhs=camoh_s[:, jc * NJ:(jc + 1) * NJ], start=True, stop=True,
                    )

                    gs = spool.tile([128, NJ], dt.float16)
                    nc.scalar.activation(out=gs, in_=g_ps, func=AF.Copy)

                    eq = spool.tile([128, NJ], dt.float16)
                    nc.vector.tensor_scalar(
                        out=eq, in0=labr, scalar1=lab_i, scalar2=None,
                        op0=AluOpType.is_equal,
                    )
                    adt = spool.tile([128, NJ], dt.float16)
                    nc.scalar.activation(out=adt, in_=tsr, func=AF.Abs, bias=nts_i, scale=1.0)
                    thg = spool.tile([128, NJ], dt.float16)
                    nc.vector.scalar_tensor_tensor(
                        out=thg, in0=th_ps, scalar=1.0, in1=eq,
                        op0=AluOpType.mult, op1=AluOpType.mult,
                    )
                    stpos = spool.tile([128, NJ], dt.float16)
                    nc.vector.scalar_tensor_tensor(
                        out=stpos, in0=adt, scalar=1.0, in1=thg,
                        op0=AluOpType.mult, op1=AluOpType.is_lt,
                        accum_out=npos_acc[:, rb, jc:jc + 1],
                    )
                    s_t = spool.tile([128, NJ], dt.float16)
                    nc.vector.scalar_tensor_tensor(
                        out=s_t, in0=gs, scalar=a_i, in1=ur,
                        op0=AluOpType.mult, op1=AluOpType.mult,
                    )
                    mc = dpool.tile([128, 1], dt.float32)
                    nc.vector.tensor_reduce(
                        out=mc, in_=s_t, axis=mybir.AxisListType.X, op=AluOpType.max,
                    )
                    iold = rb * 2 + (jc % 2)
                    inew = rb * 2 + 1 - (jc % 2)
                    m_old = mon[:, iold:iold + 1]
                    m_new = mon[:, inew:inew + 1]
                    nc.vector.tensor_tensor(out=m_new, in0=m_old, in1=mc, op=AluOpType.max)
                    negm = dpool.tile([128, 1], dt.float32)
                    nc.vector.tensor_scalar(
                        out=negm, in0=m_new, scalar1=-1.0, scalar2=None, op0=AluOpType.mult,
                    )
                    r_t = dpool.tile([128, 1], dt.float32)
                    nc.scalar.activation(out=r_t, in_=m_old, func=AF.Exp, bias=negm, scale=1.0)
                    e_t = spool.tile([128, NJ], dt.float16)
                    zc = dpool.tile([128, 1], dt.float32)
                    nc.scalar.activation(
                        out=e_t, in_=s_t, func=AF.Exp, bias=negm, scale=1.0, accum_out=zc,
                    )
                    nc.vector.scalar_tensor_tensor(
                        out=zon[:, inew:inew + 1], in0=zon[:, iold:iold + 1], scalar=r_t,
                        in1=zc, op0=AluOpType.mult, op1=AluOpType.add,
                    )
                    tq = spool.tile([128, NJ], dt.float16)
                    nc.vector.scalar_tensor_tensor(
                        out=tq, in0=gs, scalar=-2.0, in1=sqr,
                        op0=AluOpType.mult, op1=AluOpType.add,
                    )
                    v_t = spool.tile([128, NJ], dt.float16)
                    nc.gpsimd.tensor_tensor(out=v_t, in0=tq, in1=eq, op=AluOpType.mult)
                    nc.vector.tensor_reduce(
                        out=mtri_acc[:, rb, jc:jc + 1], in_=v_t,
                        axis=mybir.AxisListType.X, op=AluOpType.max,
                    )
                    pm = spool.tile([128, NJ], dt.float16)
                    nc.vector.scalar_tensor_tensor(
                        out=pm, in0=stpos, scalar=1.0, in1=s_t,
                        op0=AluOpType.mult, op1=AluOpType.mult,
                        accum_out=p_acc[:, rb, jc:jc + 1],
                    )

            # ---- classifier (id loss) ----
            for rb in range(RB):
                rsl = slice(rb * 128, (rb + 1) * 128)
                loh_t = lpool.tile([128, C], dt.float16)
                nc.sync.dma_start(out=loh_t, in_=d_loh[rb])
                lg_ps = pscls.tile([128, C], dt.float32)
                for n0, n1 in ((0, 512), (512, C)):
                    for k in range(KTA):
                        nc.tensor.matmul(
                            out=lg_ps[:, n0:n1], lhsT=fta_loc[:, k, rsl],
                            rhs=wta_s[:, k, n0:n1],
                            start=(k == 0), stop=(k == KTA - 1),
                        )
                nc.vector.tensor_reduce(
                    out=stats_s[:, rb, 5:6], in_=lg_ps, axis=mybir.AxisListType.X,
                    op=AluOpType.max, negate=True,
                )
                ecls = spool.tile([128, C], dt.float16, tag="ecls")
                nc.scalar.activation(
                    out=ecls, in_=lg_ps, func=AF.Exp, bias=stats_s[:, rb, 5:6],
                    scale=1.0, accum_out=stats_s[:, rb, 6:7],
                )
                tk = spool.tile([128, C], dt.float16, tag="ecls")
                nc.vector.scalar_tensor_tensor(
                    out=tk, in0=lg_ps, scalar=1.0, in1=loh_t,
                    op0=AluOpType.mult, op1=AluOpType.mult,
                    accum_out=stats_s[:, rb, 7:8],
                )

            # ---- gather per-row stats ----
            for rb in range(RB):
                ifin = rb * 2  # after an even number (JC=16) of chunks
                nc.vector.tensor_copy(stats_s[:, rb, 0:1], mon[:, ifin:ifin + 1])
                nc.vector.tensor_copy(stats_s[:, rb, 1:2], zon[:, ifin:ifin + 1])
                nc.vector.tensor_reduce(
                    out=stats_s[:, rb, 2:3], in_=npos_acc[:, rb, :],
                    axis=mybir.AxisListType.X, op=AluOpType.add,
                )
                nc.vector.tensor_reduce(
                    out=stats_s[:, rb, 3:4], in_=p_acc[:, rb, :],
                    axis=mybir.AxisListType.X, op=AluOpType.add,
                )
                nc.vector.tensor_reduce(
                    out=stats_s[:, rb, 4:5], in_=mtri_acc[:, rb, :],
                    axis=mybir.AxisListType.X, op=AluOpType.max,
                )
            nc.sync.dma_start(out=d_stats[:, :, :], in_=stats_s)

    nc.finalize()
    return nc


def get_nc():
    if "nc" not in _NC_CACHE:
        _NC_CACHE["nc"] = _build_nc()
    return _NC_CACHE["nc"]


def host_prep(features, labels, cameras, timestamps, reach_max, W, b):
    """Build per-core input maps + host-side helper arrays."""
    f = np.asarray(features, f32)
    labels = np.asarray(labels).astype(np.int64)
    cameras = np.asarray(cameras).astype(np.int64)
    ts = np.asarray(timestamps, f32)
    rm = np.asarray(reach_max, f32)

    fq = f.astype(f16)
    fTa = np.zeros((KTA * 128, B), f16)
    fTa[:D] = fq.T
    fTa[D] = 1.0
    fta_t = np.ascontiguousarray(
        fTa.reshape(KTA, 128, JC, NJ).transpose(0, 2, 1, 3)
    )
    wta = np.zeros((KTA, 128, C), f16)
    wta.reshape(KTA * 128, C)[:D] = np.asarray(W, f32).T.astype(f16)
    wta.reshape(KTA * 128, C)[D] = np.asarray(b, f32).astype(f16)

    sq = (f.astype(np.float64) ** 2).sum(1).astype(f32)
    u32 = (1.0 / np.sqrt(sq)).astype(f32)
    labv = labels.astype(f16)
    tsv = (ts - 1800.0).astype(f16)
    uv = u32.astype(f16)
    sqv = sq.astype(f16)
    camoh = np.zeros((NCAMS, B), f16)
    camoh[cameras, np.arange(B)] = 1.0
    rsel_full = rm[cameras]                      # [B, 16]
    loh_full = np.zeros((B, C), f16)
    loh_full[np.arange(B), labels] = 1.0
    ai32 = (u32 / TEMP).astype(f32)
    gs_ii = (fq.astype(f32) ** 2).sum(1).astype(f16)   # ≈ on-device fp16(G_ii)
    # device-replica of the diagonal sim value as it lands in the P accumulator
    # (fp32 compute, then the fp16 rounding of the s tile)
    s_ii = (gs_ii.astype(f32) * ai32 * uv.astype(f32)).astype(f16).astype(f32)

    in_maps = []
    for c in range(NCORES):
        rows = slice(c * BLOC, (c + 1) * BLOC)
        in_maps.append({
            "fta": fta_t[:KT],
            "ftaloc": np.ascontiguousarray(fta_t[:, 2 * c:2 * c + 2]),
            "wta": wta,
            "labv": labv,
            "tsv": tsv,
            "uv": uv,
            "sqv": sqv,
            "camoh": camoh,
            "rsel": np.ascontiguousarray(rsel_full[rows].T.astype(f16)),
            "loh": np.ascontiguousarray(loh_full[rows].reshape(RB, 128, C)),
            "labi": np.ascontiguousarray(labv[rows].astype(f32).reshape(RB, 128).T),
            "ntsi": np.ascontiguousarray((-tsv[rows].astype(f32)).reshape(RB, 128).T),
            "ai": np.ascontiguousarray(ai32[rows].reshape(RB, 128).T),
        })
    host = {"sq": sq, "s_ii": s_ii}
    return in_maps, host


def assemble(stats_list, host):
    """Final scalar loss from per-core per-row stats (float64 on host)."""
    rows = []
    for st in stats_list:                       # [128, RB, 8] each
        rows.append(np.transpose(np.asarray(st, np.float64), (1, 0, 2)).reshape(BLOC, 8))
    st = np.concatenate(rows, 0)                # [B, 8] in global row order
    M, Z = st[:, 0], st[:, 1]
    npos = st[:, 2] - 1.0                       # remove the diagonal pair
    P = st[:, 3] - host["s_ii"].astype(np.float64)
    mtri = st[:, 4]
    mcls = -st[:, 5]
    zcls = st[:, 6]
    take = st[:, 7]
    sq = host["sq"].astype(np.float64)

    lse = M + np.log(Z)
    npos_tot = npos.sum()
    loss_st = (npos * lse - P).sum() / npos_tot if npos_tot > 0 else 0.0

    valid = mtri > TRI_VALID_THRESH
    hardest = mtri + sq
    per_anchor = np.maximum(hardest + MARGIN, 0.0) * valid
    nv = valid.sum()
    loss_tri = per_anchor.sum() / max(nv, 1.0) if nv > 0 else 0.0

    lse_id = mcls + np.log(zcls)
    loss_id = (lse_id - take).mean()

    return np.float32(loss_id + L_TRI * loss_tri + L_ST * loss_st)


def kernel(features, labels, cameras, timestamps, reach_max, W, b):
    in_maps, host = host_prep(features, labels, cameras, timestamps, reach_max, W, b)
    nc = get_nc()
    res = run_bass_kernel_spmd(nc, in_maps, core_ids=list(range(NCORES)))
    stats_list = [res.results[c]["stats"] for c in range(NCORES)]
    return assemble(stats_list, host)


# revision 5
# speedup vs baseline: 1.1122x; 1.1122x over previous
"""Trainium2 Bass kernel for nn_STContrastiveReIDLoss (B=8192, D=2048, C=751).

Strategy (8 NeuronCores, SPMD, no collectives):
  - Shard batch rows across cores (1024 anchors/core); every core streams the
    full feature set as the RHS of a gram matmul, so all B x B pairwise
    quantities for its anchor rows are computed locally.
  - One fp16 gram matmul G = f_local @ f_all^T drives all three losses:
      * triplet:   d2_ij = sq_i + sq_j - 2 G_ij   (hardest_neg == 0 analytically,
                   since the reference's neg mask keeps the diagonal and d2_ii = 0)
      * st-InfoNCE: sim_ij = G_ij * u_i * u_j / TEMP  (u = 1/||f||)
      * id loss:    separate fp16 matmul vs W^T (bias folded in via K+1 row)
  - |sim| <= 1/TEMP by Cauchy-Schwarz, so the softmax max is the constant
    1/TEMP: no row-max pass, no online rescaling; exp sums accumulate per
    column-chunk straight from the activation engine's accumulator.
  - Label equality masks via fp16 compares on-chip; camera reachability
    threshold via a K=16 one-hot matmul th_ij = reach_max[cam_i, cam_j].
  - Per-row partial stats are written out ([128, 8, 8] fp32 per core); the
    final scalar reduction (logs, divisions, diagonal corrections) runs on
    host in float64.
"""

import numpy as np

import concourse.bacc as bacc
import concourse.bass as bass
import concourse.mybir as mybir
from concourse.alu_op_type import AluOpType
from concourse.bass_utils import run_bass_kernel_spmd
from concourse.tile import TileContext

B, D, C = 8192, 2048, 751
NCAMS = 16
MARGIN = 0.3
TEMP = 0.07
L_TRI = 0.5
L_ST = 0.3

NCORES = 8
BLOC = B // NCORES          # rows per core (1024)
RB = BLOC // 128            # row-blocks per core (8)
NJ = 512                    # column chunk width
JC = B // NJ                # column chunks (16)
KT = D // 128               # contraction k-tiles (16)
KTA = KT + 1                # + bias row tile for the classifier
M0 = float(np.float32(1.0 / TEMP))   # exact softmax max bound (Cauchy-Schwarz)
TRI_VALID_THRESH = 1000.0   # mtri above this => anchor has a real positive

f16 = np.float16
f32 = np.float32
dt = mybir.dt
AF = mybir.ActivationFunctionType

_NC_CACHE = {}


def _build_nc():
    nc = bacc.Bacc("TRN2", target_bir_lowering=False, debug=False)

    d_fta = nc.dram_tensor("fta", [KT, JC, 128, NJ], dt.float16, kind="ExternalInput")
    d_ftaloc = nc.dram_tensor("ftaloc", [KTA, 2, 128, NJ], dt.float16, kind="ExternalInput")
    d_wta = nc.dram_tensor("wta", [KTA, 128, C], dt.float16, kind="ExternalInput")
    d_labv = nc.dram_tensor("labv", [B], dt.float16, kind="ExternalInput")
    d_tsv = nc.dram_tensor("tsv", [B], dt.float16, kind="ExternalInput")
    d_uv = nc.dram_tensor("uv", [B], dt.float16, kind="ExternalInput")
    d_sqv = nc.dram_tensor("sqv", [B], dt.float16, kind="ExternalInput")
    d_camoh = nc.dram_tensor("camoh", [NCAMS, B], dt.float16, kind="ExternalInput")
    d_rsel = nc.dram_tensor("rsel", [NCAMS, BLOC], dt.float16, kind="ExternalInput")
    d_loh = nc.dram_tensor("loh", [RB, 128, C], dt.float16, kind="ExternalInput")
    d_labi = nc.dram_tensor("labi", [128, RB], dt.float32, kind="ExternalInput")
    d_ntsi = nc.dram_tensor("ntsi", [128, RB], dt.float32, kind="ExternalInput")
    d_ai = nc.dram_tensor("ai", [128, RB], dt.float32, kind="ExternalInput")
    d_stats = nc.dram_tensor("stats", [128, RB, 8], dt.float32, kind="ExternalOutput")

    def bcast(dram_vec, off, n):
        return bass.AP(tensor=dram_vec, offset=off, ap=[[0, 128], [1, n]])

    with TileContext(nc) as tc:
        with (
            tc.tile_pool(name="const", bufs=1) as cpool,
            tc.tile_pool(name="accs", bufs=1) as apool,
            tc.tile_pool(name="rhs", bufs=2) as rpool,
            tc.tile_pool(name="vecs", bufs=2) as vpool,
            tc.tile_pool(name="loh", bufs=2) as lpool,
            tc.tile_pool(name="scr", bufs=3) as spool,
            tc.tile_pool(name="side", bufs=6) as dpool,
            tc.tile_pool(name="psg", bufs=3, space="PSUM") as psg,
            tc.tile_pool(name="psth", bufs=2, space="PSUM") as psth,
            tc.tile_pool(name="pscls", bufs=1, space="PSUM") as pscls,
        ):
            # ---- resident constants ----
            fta_loc = cpool.tile([128, KTA, BLOC], dt.float16)
            for k in range(KTA):
                for h in range(2):
                    nc.sync.dma_start(
                        out=fta_loc[:, k, h * NJ:(h + 1) * NJ], in_=d_ftaloc[k, h]
                    )
            wta_s = cpool.tile([128, KTA, C], dt.float16)
            for k in range(KTA):
                nc.sync.dma_start(out=wta_s[:, k, :], in_=d_wta[k])
            camoh_s = cpool.tile([NCAMS, B], dt.float16)
            nc.sync.dma_start(out=camoh_s, in_=d_camoh[:, :])
            rsel_s = cpool.tile([NCAMS, BLOC], dt.float16)
            nc.sync.dma_start(out=rsel_s, in_=d_rsel[:, :])
            labi_s = cpool.tile([128, RB], dt.float32)
            nc.sync.dma_start(out=labi_s, in_=d_labi[:, :])
            ntsi_s = cpool.tile([128, RB], dt.float32)
            nc.sync.dma_start(out=ntsi_s, in_=d_ntsi[:, :])
            ai_s = cpool.tile([128, RB], dt.float32)
            nc.sync.dma_start(out=ai_s, in_=d_ai[:, :])
            negm0 = cpool.tile([128, 1], dt.float32)
            nc.vector.memset(negm0, -M0)

            # ---- accumulators ----
            npos_acc = apool.tile([128, RB, JC], dt.float32)
            p_acc = apool.tile([128, RB, JC], dt.float32)
            mtri_acc = apool.tile([128, RB, JC], dt.float32)
            z_acc = apool.tile([128, RB, JC], dt.float32)
            stats_s = apool.tile([128, RB, 8], dt.float32)

            # ---- main loop over column chunks ----
            for jc in range(JC):
                rhs_t = rpool.tile([128, KT, NJ], dt.float16)
                for k in range(KT):
                    nc.sync.dma_start(out=rhs_t[:, k, :], in_=d_fta[k, jc])
                labr = vpool.tile([128, NJ], dt.float16)
                nc.sync.dma_start(out=labr, in_=bcast(d_labv, jc * NJ, NJ))
                tsr = vpool.tile([128, NJ], dt.float16)
                nc.sync.dma_start(out=tsr, in_=bcast(d_tsv, jc * NJ, NJ))
                ur = vpool.tile([128, NJ], dt.float16)
                nc.sync.dma_start(out=ur, in_=bcast(d_uv, jc * NJ, NJ))
                sqr = vpool.tile([128, NJ], dt.float16)
                nc.sync.dma_start(out=sqr, in_=bcast(d_sqv, jc * NJ, NJ))

                for rb in range(RB):
                    rsl = slice(rb * 128, (rb + 1) * 128)
                    lab_i = labi_s[:, rb:rb + 1]
                    nts_i = ntsi_s[:, rb:rb + 1]
                    a_i = ai_s[:, rb:rb + 1]

                    g_ps = psg.tile([128, NJ], dt.float32)
                    for k in range(KT):
                        nc.tensor.matmul(
                            out=g_ps, lhsT=fta_loc[:, k, rsl], rhs=rhs_t[:, k, :],
                            start=(k == 0), stop=(k == KT - 1),
                        )
                    th_ps = psth.tile([128, NJ], dt.float32)
                    nc.tensor.matmul(
                        out=th_ps, lhsT=rsel_s[:, rsl],
                        rhs=camoh_s[:, jc * NJ:(jc + 1) * NJ], start=True, stop=True,
                    )

                    # masks
                    eq = spool.tile([128, NJ], dt.float16)
                    nc.vector.tensor_scalar(
                        out=eq, in0=labr, scalar1=lab_i, scalar2=None,
                        op0=AluOpType.is_equal,
                    )
                    adt = spool.tile([128, NJ], dt.float16)
                    nc.scalar.activation(out=adt, in_=tsr, func=AF.Abs, bias=nts_i, scale=1.0)
                    thg = spool.tile([128, NJ], dt.float16)
                    nc.vector.tensor_tensor(out=thg, in0=th_ps, in1=eq, op=AluOpType.mult)
                    stpos = spool.tile([128, NJ], dt.float16)
                    nc.vector.scalar_tensor_tensor(
                        out=stpos, in0=adt, scalar=1.0, in1=thg,
                        op0=AluOpType.mult, op1=AluOpType.is_lt,
                        accum_out=npos_acc[:, rb, jc:jc + 1],
                    )
                    # similarity + exp-sum (constant max bound M0)
                    s_t = spool.tile([128, NJ], dt.float16)
                    nc.vector.scalar_tensor_tensor(
                        out=s_t, in0=g_ps, scalar=a_i, in1=ur,
                        op0=AluOpType.mult, op1=AluOpType.mult,
                    )
                    e_t = spool.tile([128, NJ], dt.float16)
                    nc.scalar.activation(
                        out=e_t, in_=s_t, func=AF.Exp, bias=negm0, scale=1.0,
                        accum_out=z_acc[:, rb, jc:jc + 1],
                    )
                    pm = spool.tile([128, NJ], dt.float16)
                    nc.vector.scalar_tensor_tensor(
                        out=pm, in0=stpos, scalar=1.0, in1=s_t,
                        op0=AluOpType.mult, op1=AluOpType.mult,
                        accum_out=p_acc[:, rb, jc:jc + 1],
                    )
                    # triplet hardest-positive surrogate
                    gm2 = spool.tile([128, NJ], dt.float16)
                    nc.scalar.activation(out=gm2, in_=g_ps, func=AF.Copy, scale=-2.0)
                    tq = spool.tile([128, NJ], dt.float16)
                    nc.gpsimd.tensor_tensor(out=tq, in0=gm2, in1=sqr, op=AluOpType.add)
                    v_t = spool.tile([128, NJ], dt.float16)
                    nc.gpsimd.tensor_tensor(out=v_t, in0=tq, in1=eq, op=AluOpType.mult)
                    nc.vector.tensor_reduce(
                        out=mtri_acc[:, rb, jc:jc + 1], in_=v_t,
                        axis=mybir.AxisListType.X, op=AluOpType.max,
                    )

            # ---- classifier (id loss) ----
            for rb in range(RB):
                rsl = slice(rb * 128, (rb + 1) * 128)
                loh_t = lpool.tile([128, C], dt.float16)
                nc.sync.dma_start(out=loh_t, in_=d_loh[rb])
                lg_ps = pscls.tile([128, C], dt.float32)
                for n0, n1 in ((0, 512), (512, C)):
                    for k in range(KTA):
                        nc.tensor.matmul(
                            out=lg_ps[:, n0:n1], lhsT=fta_loc[:, k, rsl],
                            rhs=wta_s[:, k, n0:n1],
                            start=(k == 0), stop=(k == KTA - 1),
                        )
                nc.vector.tensor_reduce(
                    out=stats_s[:, rb, 5:6], in_=lg_ps, axis=mybir.AxisListType.X,
                    op=AluOpType.max, negate=True,
                )
                ecls = spool.tile([128, C], dt.float16, tag="ecls")
                nc.scalar.activation(
                    out=ecls, in_=lg_ps, func=AF.Exp, bias=stats_s[:, rb, 5:6],
                    scale=1.0, accum_out=stats_s[:, rb, 6:7],
                )
                tk = spool.tile([128, C], dt.float16, tag="ecls")
                nc.vector.scalar_tensor_tensor(
                    out=tk, in0=lg_ps, scalar=1.0, in1=loh_t,
                    op0=AluOpType.mult, op1=AluOpType.mult,
                    accum_out=stats_s[:, rb, 7:8],
                )

            # ---- gather per-row stats ----
            for rb in range(RB):
                nc.vector.tensor_reduce(
                    out=stats_s[:, rb, 0:1], in_=z_acc[:, rb, :],
                    axis=mybir.AxisListType.X, op=AluOpType.add,
                )
                nc.vector.tensor_reduce(
                    out=stats_s[:, rb, 2:3], in_=npos_acc[:, rb, :],
                    axis=mybir.AxisListType.X, op=AluOpType.add,
                )
                nc.vector.tensor_reduce(
                    out=stats_s[:, rb, 3:4], in_=p_acc[:, rb, :],
                    axis=mybir.AxisListType.X, op=AluOpType.add,
                )
                nc.vector.tensor_reduce(
                    out=stats_s[:, rb, 4:5], in_=mtri_acc[:, rb, :],
                    axis=mybir.AxisListType.X, op=AluOpType.max,
                )
                nc.vector.memset(stats_s[:, rb, 1:2], 0.0)
            nc.sync.dma_start(out=d_stats[:, :, :], in_=stats_s)

    nc.finalize()
    return nc


def get_nc():
    if "nc" not in _NC_CACHE:
        _NC_CACHE["nc"] = _build_nc()
    return _NC_CACHE["nc"]


def host_prep(features, labels, cameras, timestamps, reach_max, W, b):
    """Build per-core input maps + host-side helper arrays."""
    f = np.asarray(features, f32)
    labels = np.asarray(labels).astype(np.int64)
    cameras = np.asarray(cameras).astype(np.int64)
    ts = np.asarray(timestamps, f32)
    rm = np.asarray(reach_max, f32)

    fq = f.astype(f16)
    fTa = np.zeros((KTA * 128, B), f16)
    fTa[:D] = fq.T
    fTa[D] = 1.0
    fta_t = np.ascontiguousarray(
        fTa.reshape(KTA, 128, JC, NJ).transpose(0, 2, 1, 3)
    )
    wta = np.zeros((KTA, 128, C), f16)
    wta.reshape(KTA * 128, C)[:D] = np.asarray(W, f32).T.astype(f16)
    wta.reshape(KTA * 128, C)[D] = np.asarray(b, f32).astype(f16)

    sq = (f.astype(np.float64) ** 2).sum(1).astype(f32)
    u32 = (1.0 / np.sqrt(sq)).astype(f32)
    labv = labels.astype(f16)
    tsv = (ts - 1800.0).astype(f16)
    uv = u32.astype(f16)
    sqv = sq.astype(f16)
    camoh = np.zeros((NCAMS, B), f16)
    camoh[cameras, np.arange(B)] = 1.0
    rsel_full = rm[cameras]                      # [B, 16]
    loh_full = np.zeros((B, C), f16)
    loh_full[np.arange(B), labels] = 1.0
    ai32 = (u32 / TEMP).astype(f32)
    # device-replica of the diagonal sim contribution to the P accumulator:
    # fp32 (psum) G_ii approx, then the stt's internal fp32 product
    sqpe32 = (fq.astype(f32) ** 2).sum(1).astype(f32)
    s_ii = sqpe32 * ai32 * uv.astype(f32)

    in_maps = []
    for c in range(NCORES):
        rows = slice(c * BLOC, (c + 1) * BLOC)
        in_maps.append({
            "fta": fta_t[:KT],
            "ftaloc": np.ascontiguousarray(fta_t[:, 2 * c:2 * c + 2]),
            "wta": wta,
            "labv": labv,
            "tsv": tsv,
            "uv": uv,
            "sqv": sqv,
            "camoh": camoh,
            "rsel": np.ascontiguousarray(rsel_full[rows].T.astype(f16)),
            "loh": np.ascontiguousarray(loh_full[rows].reshape(RB, 128, C)),
            "labi": np.ascontiguousarray(labv[rows].astype(f32).reshape(RB, 128).T),
            "ntsi": np.ascontiguousarray((-tsv[rows].astype(f32)).reshape(RB, 128).T),
            "ai": np.ascontiguousarray(ai32[rows].reshape(RB, 128).T),
        })
    host = {"sq": sq, "s_ii": s_ii}
    return in_maps, host


def assemble(stats_list, host):
    """Final scalar loss from per-core per-row stats (float64 on host)."""
    rows = []
    for st in stats_list:                       # [128, RB, 8] each
        rows.append(np.transpose(np.asarray(st, np.float64), (1, 0, 2)).reshape(BLOC, 8))
    st = np.concatenate(rows, 0)                # [B, 8] in global row order
    Z = st[:, 0]
    npos = st[:, 2] - 1.0                       # remove the diagonal pair
    P = st[:, 3] - host["s_ii"].astype(np.float64)
    mtri = st[:, 4]
    mcls = -st[:, 5]
    zcls = st[:, 6]
    take = st[:, 7]
    sq = host["sq"].astype(np.float64)

    lse = M0 + np.log(Z)
    npos_tot = npos.sum()
    loss_st = (npos * lse - P).sum() / npos_tot if npos_tot > 0 else 0.0

    valid = mtri > TRI_VALID_THRESH
    hardest = mtri + sq
    per_anchor = np.maximum(hardest + MARGIN, 0.0) * valid
    nv = valid.sum()
    loss_tri = per_anchor.sum() / max(nv, 1.0) if nv > 0 else 0.0

    lse_id = mcls + np.log(zcls)
    loss_id = (lse_id - take).mean()

    return np.float32(loss_id + L_TRI * loss_tri + L_ST * loss_st)


def kernel(features, labels, cameras, timestamps, reach_max, W, b):
    in_maps, host = host_prep(features, labels, cameras, timestamps, reach_max, W, b)
    nc = get_nc()
    res = run_bass_kernel_spmd(nc, in_maps, core_ids=list(range(NCORES)))
    stats_list = [res.results[c]["stats"] for c in range(NCORES)]
    return assemble(stats_list, host)
